# revision 1
# baseline (speedup 1.0000x reference)
"""Trainium2 Bass kernel for ContinuousFilterConvolution (SchNet cfconv).

out[a, :] = sum_{e: seg_i[e]=a} filters(d_e) * atom_features[idx_j[e], :]
filters(d) = ssp(ssp(rbf(d) @ W1 + b1) @ W2 + b2), ssp = softplus - log 2.

Strategy (8 NeuronCores, full inputs in / full output out):
- Atoms are partitioned contiguously across cores (seg_i is sorted, so each
  core owns a contiguous edge range; no cross-core reduction needed).
- Per core, destination atoms are grouped into 128-atom blocks. Each block's
  edges are split by source-atom half (idx_j < 25024 vs >=) so gather indices
  fit int16, and padded to a fixed slot count (static SPMD program).
- atom_features rows (fp16) are gathered edge-major by nc.gpsimd.dma_gather.
- filters(d) is a 1-D function of distance: approximated by a sigmoid-basis
  expansion fitted on the host from the runtime weights (max err ~4e-5 vs
  an output scale of ~0.14). On-device: PE broadcast-matmul of d (hi+lo fp16
  split) -> ACT sigmoid with per-partition scale/bias -> PE basis matmul
  -> PE transposes to edge-major.
- x = feat * filters on DVE; scatter-add via PE matmul with one-hot
  selection matrices (built by iota-compare on DVE) accumulating into a
  per-block PSUM tile; per-block copy-out to DRAM.
"""
import numpy as np

N_CORES = 8
NAT = 50000
E = 800000
D = 128
ATB = 128             # atoms per scatter block
APC = NAT // N_CORES  # atoms per core (6250)
NBLK = (APC + ATB - 1) // ATB  # 49 blocks/core
SPLIT = 25024         # source-atom half split (int16 index limit)
M = 64                # basis size (63 sigmoids + 1 const)

_cache = {}


def _fit_basis(distances, centers, gamma, W1, b1, W2, b2):
    """Fit filters(d) ~ C.T @ sigmoid(d*s + b) on the host. Returns
    (scale [M], bias [M], C [M, D] fp32, fit report)."""
    dmin = float(distances.min())
    dmax = float(distances.max())
    span = max(dmax - dmin, 1e-6)
    t = np.linspace(dmin - 0.05 * span, dmax + 0.05 * span, M - 1).astype(np.float64)
    w = (t[1] - t[0])
    scale = np.full(M, 1.0 / w, np.float64)
    bias = -t / w
    # constant basis element
    scale = np.concatenate([scale[: M - 1], [0.0]])
    bias = np.concatenate([bias[: M - 1], [20.0]])

    dg = np.linspace(dmin, dmax, 8192).astype(np.float64)

    def F(d):
        e = np.exp(-gamma[None, :].astype(np.float64)
                   * (d[:, None] - centers[None, :].astype(np.float64)) ** 2)
        h1 = np.logaddexp(0, e @ W1.astype(np.float64) + b1) - np.log(2.0)
        return np.logaddexp(0, h1 @ W2.astype(np.float64) + b2) - np.log(2.0)

    Phi = 1.0 / (1.0 + np.exp(-(dg[:, None] * scale[None, :] + bias[None, :])))
    Y = F(dg)
    C, *_ = np.linalg.lstsq(Phi, Y, rcond=None)
    err = np.abs(Phi @ C - Y).max()
    return (scale.astype(np.float32), bias.astype(np.float32),
            C.astype(np.float32), err)


def _build_nc(slots_half):
    import concourse.bacc as bacc
    import concourse.mybir as mybir
    import concourse.tile as tile

    fp16 = mybir.dt.float16
    fp32 = mybir.dt.float32
    i16 = mybir.dt.int16

    nhalf = 2 * NBLK
    nst = slots_half // 128   # sub-tiles (128 slots) per half
    W = slots_half // 16      # idx columns per half

    nc = bacc.Bacc(None, target_bir_lowering=False, debug=False)
    tbl_lo = nc.declare_dram_parameter("tbl_lo", [SPLIT, D], fp16, isOutput=False)
    tbl_hi = nc.declare_dram_parameter("tbl_hi", [NAT - SPLIT, D], fp16, isOutput=False)
    idx_d = nc.declare_dram_parameter("idx", [128, nhalf * W], i16, isOutput=False)
    seg_d = nc.declare_dram_parameter("seg", [128, nhalf * nst], fp32, isOutput=False)
    dhi_d = nc.declare_dram_parameter("dhi", [nhalf, slots_half], fp16, isOutput=False)
    dlo_d = nc.declare_dram_parameter("dlo", [nhalf, slots_half], fp16, isOutput=False)
    ones_d = nc.declare_dram_parameter("ones", [1, M], fp16, isOutput=False)
    ksc_d = nc.declare_dram_parameter("ksc", [M, 1], fp32, isOutput=False)
    kbi_d = nc.declare_dram_parameter("kbi", [M, 1], fp32, isOutput=False)
    C_d = nc.declare_dram_parameter("C", [M, D], fp16, isOutput=False)
    ident_d = nc.declare_dram_parameter("ident", [128, 128], fp16, isOutput=False)
    iota_d = nc.declare_dram_parameter("iota", [128, ATB], fp16, isOutput=False)
    out_d = nc.declare_dram_parameter("out", [128, NBLK * ATB], fp32, isOutput=True)

    with tile.TileContext(nc) as tc:
        with (
            tc.tile_pool(name="const", bufs=1) as cpool,
            tc.tile_pool(name="gat", bufs=4) as gpool,
            tc.tile_pool(name="dist", bufs=3) as dpool,
            tc.tile_pool(name="phis", bufs=2) as phpool,
            tc.tile_pool(name="filts", bufs=2) as fspool,
            tc.tile_pool(name="xs", bufs=2) as xpool,
            tc.tile_pool(name="ssel", bufs=4) as spool,
            tc.tile_pool(name="osb", bufs=2) as opool,
            tc.tile_pool(name="pbig", bufs=1, space="PSUM") as pbig,
            tc.tile_pool(name="pfe", bufs=1, space="PSUM") as pfe,
            tc.tile_pool(name="pout", bufs=2, space="PSUM") as pout,
        ):
            idx_sb = cpool.tile([128, nhalf * W], i16)
            nc.sync.dma_start(idx_sb[:], idx_d[:])
            seg_sb = cpool.tile([128, nhalf * nst], fp32)
            nc.sync.dma_start(seg_sb[:], seg_d[:])
            ones_sb = cpool.tile([1, M], fp16)
            nc.sync.dma_start(ones_sb[:], ones_d[:])
            ksc_sb = cpool.tile([M, 1], fp32)
            nc.sync.dma_start(ksc_sb[:], ksc_d[:])
            kbi_sb = cpool.tile([M, 1], fp32)
            nc.sync.dma_start(kbi_sb[:], kbi_d[:])
            C_sb = cpool.tile([M, D], fp16)
            nc.sync.dma_start(C_sb[:], C_d[:])
            ident_sb = cpool.tile([128, 128], fp16)
            nc.sync.dma_start(ident_sb[:], ident_d[:])
            iota_sb = cpool.tile([128, ATB], fp16)
            nc.sync.dma_start(iota_sb[:], iota_d[:])

            for blk in range(NBLK):
                out_ps = pout.tile([128, ATB], fp32, tag="outp")
                for half in range(2):
                    h = blk * 2 + half
                    src = tbl_lo if half == 0 else tbl_hi
                    g = gpool.tile([128, nst * D], fp16, tag="g")
                    nc.gpsimd.dma_gather(
                        out_ap=g[:].rearrange("p (n d) -> p n d", d=D),
                        in_ap=src[:],
                        idxs_ap=idx_sb[:, h * W:(h + 1) * W],
                        num_idxs=slots_half,
                        num_idxs_reg=slots_half,
                        elem_size=D,
                        single_packet=False,
                    )
                    dh = dpool.tile([1, slots_half], fp16, tag="dh")
                    nc.sync.dma_start(dh[:], dhi_d[h:h + 1, :])
                    dl = dpool.tile([1, slots_half], fp16, tag="dl")
                    nc.sync.dma_start(dl[:], dlo_d[h:h + 1, :])

                    phi_ps = pbig.tile([128, slots_half], fp32, tag="big")
                    for c0 in range(0, slots_half, 512):
                        c1 = min(c0 + 512, slots_half)
                        nc.tensor.matmul(phi_ps[:M, c0:c1], ones_sb[:],
                                         dh[:, c0:c1], start=True, stop=False)
                        nc.tensor.matmul(phi_ps[:M, c0:c1], ones_sb[:],
                                         dl[:, c0:c1], start=False, stop=True)
                    phi_sb = phpool.tile([128, slots_half], fp16, tag="phi")
                    nc.scalar.activation(
                        phi_sb[:M, :], phi_ps[:M, :],
                        mybir.ActivationFunctionType.Sigmoid,
                        bias=kbi_sb[:], scale=ksc_sb[:])

                    filt_ps = pbig.tile([128, slots_half], fp32, tag="big")
                    for c0 in range(0, slots_half, 512):
                        c1 = min(c0 + 512, slots_half)
                        nc.tensor.matmul(filt_ps[:, c0:c1], C_sb[:],
                                         phi_sb[:M, c0:c1], start=True, stop=True)
                    filt_sb = fspool.tile([128, slots_half], fp16, tag="filt")
                    nc.scalar.activation(filt_sb[:], filt_ps[:],
                                         mybir.ActivationFunctionType.Copy)

                    fe_ps = pfe.tile([128, slots_half], fp16, tag="fe")
                    for j in range(nst):
                        nc.tensor.transpose(fe_ps[:, j * 128:(j + 1) * 128],
                                            filt_sb[:, j * 128:(j + 1) * 128],
                                            ident_sb[:])
                    x_sb = xpool.tile([128, nst * D], fp16, tag="x")
                    nc.vector.tensor_mul(x_sb[:], g[:], fe_ps[:])

                    for j in range(nst):
                        S_sb = spool.tile([128, ATB], fp16, tag="S")
                        nc.vector.tensor_scalar(
                            out=S_sb[:], in0=iota_sb[:],
                            scalar1=seg_sb[:, h * nst + j:h * nst + j + 1],
                            scalar2=None,
                            op0=mybir.AluOpType.is_equal)
                        nc.tensor.matmul(
                            out_ps[:], x_sb[:, j * D:(j + 1) * D], S_sb[:],
                            start=(half == 0 and j == 0),
                            stop=(half == 1 and j == nst - 1),
                            skip_group_check=True)
                o_sb = opool.tile([128, ATB], fp32, tag="o")
                nc.vector.tensor_copy(o_sb[:], out_ps[:])
                nc.sync.dma_start(out_d[:, blk * ATB:(blk + 1) * ATB], o_sb[:])
    nc.compile()
    return nc


def _get_nc(slots_half):
    if slots_half not in _cache:
        _cache[slots_half] = _build_nc(slots_half)
    return _cache[slots_half]


def kernel(atom_features, distances, idx_j, seg_i, centers, gamma,
           W1, b1, W2, b2):
    from concourse.bass_utils import run_bass_kernel_spmd

    atom_features = np.asarray(atom_features, np.float32)
    distances = np.asarray(distances, np.float32)
    idx_j = np.asarray(idx_j, np.int32)
    seg_i = np.asarray(seg_i, np.int32)
    centers = np.asarray(centers, np.float32)
    gamma = np.asarray(gamma, np.float32)
    W1 = np.asarray(W1, np.float32)
    b1 = np.asarray(b1, np.float32)
    W2 = np.asarray(W2, np.float32)
    b2 = np.asarray(b2, np.float32)

    ksc, kbi, C, fit_err = _fit_basis(distances, centers, gamma, W1, b1, W2, b2)

    feat16 = atom_features.astype(np.float16)
    d16 = distances.astype(np.float16)
    dlo16 = (distances - d16.astype(np.float32)).astype(np.float16)

    # per-core, per-block, per-half slot assignment
    order = np.arange(E)  # seg_i already sorted; edges in seg order
    core = seg_i // APC
    segc = seg_i - core * APC
    blk = segc // ATB
    segb = (segc % ATB).astype(np.float32)
    is_lo = idx_j < SPLIT

    # max slots per (core, block, half)
    keys = (core.astype(np.int64) * NBLK + blk) * 2 + (~is_lo)
    cnt = np.bincount(keys, minlength=N_CORES * NBLK * 2)
    slots_half = max(1152, int(-(-cnt.max() // 128) * 128))
    nst = slots_half // 128
    nhalf = 2 * NBLK
    Wc = slots_half // 16

    nc = _get_nc(slots_half)

    # build per-core input arrays
    in_maps = []
    # slot position within each (core, blk, half)
    order_k = np.argsort(keys, kind="stable")
    pos_sorted = np.arange(E) - np.repeat(np.cumsum(cnt) - cnt, cnt)
    pos = np.empty(E, np.int64)
    pos[order_k] = pos_sorted
    # token id within half = pos; sub-tile j = pos//128, partition p = pos%128
    for c in range(N_CORES):
        idx_arr = np.zeros((nhalf, slots_half), np.int16)
        seg_arr = np.full((128, nhalf * nst), -1.0, np.float32)
        dhi_arr = np.zeros((nhalf, slots_half), np.float16)
        dlo_arr = np.zeros((nhalf, slots_half), np.float16)
        m = core == c
        hh = blk[m] * 2 + (~is_lo[m])
        pp = pos[m]
        src_idx = np.where(is_lo[m], idx_j[m], idx_j[m] - SPLIT).astype(np.int16)
        idx_arr[hh, pp] = src_idx
        seg_arr[pp % 128, hh * nst + pp // 128] = segb[m]
        dhi_arr[hh, pp] = d16[m]
        dlo_arr[hh, pp] = dlo16[m]
        # wrap idx: token i -> partition i%16, col i//16, replicated x8
        idx_wrap = np.ascontiguousarray(
            idx_arr.reshape(nhalf, Wc, 16).transpose(2, 0, 1).reshape(16, nhalf * Wc))
        idx_wrap = np.tile(idx_wrap, (8, 1))
        in_maps.append({
            "tbl_lo": feat16[:SPLIT],
            "tbl_hi": feat16[SPLIT:],
            "idx": idx_wrap,
            "seg": seg_arr,
            "dhi": dhi_arr,
            "dlo": dlo_arr,
            "ones": np.ones((1, M), np.float16),
            "ksc": ksc.reshape(M, 1),
            "kbi": kbi.reshape(M, 1),
            "C": C.astype(np.float16),
            "ident": np.eye(128, dtype=np.float16),
            "iota": np.tile(np.arange(ATB, dtype=np.float16), (128, 1)),
        })

    res = run_bass_kernel_spmd(nc, in_maps, list(range(N_CORES)))
    out = np.empty((NAT, D), np.float32)
    for c in range(N_CORES):
        out[c * APC:(c + 1) * APC] = res.results[c]["out"][:, :APC].T
    return out



# revision 2
# speedup vs baseline: 1.0264x; 1.0264x over previous
"""Trainium2 Bass kernel for ContinuousFilterConvolution (SchNet cfconv).

out[a, :] = sum_{e: seg_i[e]=a} filters(d_e) * atom_features[idx_j[e], :]
filters(d) = ssp(ssp(rbf(d) @ W1 + b1) @ W2 + b2), ssp = softplus - log 2.

Strategy (8 NeuronCores, full inputs in / full output out):
- seg_i is sorted, so cores take contiguous atom ranges balanced by EDGE
  count (atom-aligned cuts), eliminating cross-core reduction.
- Per core, edges are greedily packed into blocks: a block closes when
  either source-half reaches SLOTS_HALF edges or its dest-atom span would
  exceed ATB=128.  Blocks are edge-balanced (few % padding) instead of
  atom-aligned (13% padding) -- the kernel is gpsimd descriptor-generation
  bound (~8.2 ns per gathered row), so padding is pure loss.
- Edges are split by source-atom half (idx_j < 25024 vs >=) so gather
  indices fit int16; atom_features rows (fp16) are gathered edge-major by
  nc.gpsimd.dma_gather.
- filters(d) is a 1-D function of distance: approximated by a sigmoid-basis
  expansion fitted on the host from the runtime weights (max err ~4e-5 vs
  an output scale of ~0.14). On-device: PE broadcast-matmul of d (hi+lo fp16
  split) -> ACT sigmoid with per-partition scale/bias -> PE basis matmul
  -> PE transposes to edge-major.
- x = feat * filters on DVE; scatter-add via PE matmul with one-hot
  selection matrices (built by iota-compare on DVE) accumulating into a
  per-block PSUM tile; per-block copy-out to a disjoint DRAM scratch,
  reassembled (with overlap adds at block seams) on the host.
"""
import numpy as np

N_CORES = 8
NAT = 50000
E = 800000
D = 128
ATB = 128             # max dest-atom span of a block (S one-hot columns)
SPLIT = 25024         # source-atom half split (int16 index limit)
M = 64                # basis size (63 sigmoids + 1 const)
SLOTS_HALF = 640      # slots per (block, half); multiple of 128

_cache = {}


def _fit_basis(distances, centers, gamma, W1, b1, W2, b2):
    """Fit filters(d) ~ C.T @ sigmoid(d*s + b) on the host. Returns
    (scale [M], bias [M], C [M, D] fp32, fit report)."""
    dmin = float(distances.min())
    dmax = float(distances.max())
    span = max(dmax - dmin, 1e-6)
    t = np.linspace(dmin - 0.05 * span, dmax + 0.05 * span, M - 1).astype(np.float64)
    w = (t[1] - t[0])
    scale = np.full(M, 1.0 / w, np.float64)
    bias = -t / w
    # constant basis element
    scale = np.concatenate([scale[: M - 1], [0.0]])
    bias = np.concatenate([bias[: M - 1], [20.0]])

    dg = np.linspace(dmin, dmax, 8192).astype(np.float64)

    def F(d):
        e = np.exp(-gamma[None, :].astype(np.float64)
                   * (d[:, None] - centers[None, :].astype(np.float64)) ** 2)
        h1 = np.logaddexp(0, e @ W1.astype(np.float64) + b1) - np.log(2.0)
        return np.logaddexp(0, h1 @ W2.astype(np.float64) + b2) - np.log(2.0)

    Phi = 1.0 / (1.0 + np.exp(-(dg[:, None] * scale[None, :] + bias[None, :])))
    Y = F(dg)
    C, *_ = np.linalg.lstsq(Phi, Y, rcond=None)
    err = np.abs(Phi @ C - Y).max()
    return (scale.astype(np.float32), bias.astype(np.float32),
            C.astype(np.float32), err)


def _build_nc(slots_half, nblk):
    import concourse.bacc as bacc
    import concourse.mybir as mybir
    import concourse.tile as tile

    fp16 = mybir.dt.float16
    fp32 = mybir.dt.float32
    i16 = mybir.dt.int16

    nhalf = 2 * nblk
    nst = slots_half // 128   # sub-tiles (128 slots) per half
    W = slots_half // 16      # idx columns per half

    nc = bacc.Bacc(None, target_bir_lowering=False, debug=False)
    tbl_lo = nc.declare_dram_parameter("tbl_lo", [SPLIT, D], fp16, isOutput=False)
    tbl_hi = nc.declare_dram_parameter("tbl_hi", [NAT - SPLIT, D], fp16, isOutput=False)
    idx_d = nc.declare_dram_parameter("idx", [128, nhalf * W], i16, isOutput=False)
    seg_d = nc.declare_dram_parameter("seg", [128, nhalf * nst], fp32, isOutput=False)
    dhi_d = nc.declare_dram_parameter("dhi", [nhalf, slots_half], fp16, isOutput=False)
    dlo_d = nc.declare_dram_parameter("dlo", [nhalf, slots_half], fp16, isOutput=False)
    ones_d = nc.declare_dram_parameter("ones", [1, M], fp16, isOutput=False)
    ksc_d = nc.declare_dram_parameter("ksc", [M, 1], fp32, isOutput=False)
    kbi_d = nc.declare_dram_parameter("kbi", [M, 1], fp32, isOutput=False)
    C_d = nc.declare_dram_parameter("C", [M, D], fp16, isOutput=False)
    ident_d = nc.declare_dram_parameter("ident", [128, 128], fp16, isOutput=False)
    iota_d = nc.declare_dram_parameter("iota", [128, ATB], fp16, isOutput=False)
    out_d = nc.declare_dram_parameter("out", [128, nblk * ATB], fp32, isOutput=True)

    with tile.TileContext(nc) as tc:
        with (
            tc.tile_pool(name="const", bufs=1) as cpool,
            tc.tile_pool(name="gat", bufs=4) as gpool,
            tc.tile_pool(name="dist", bufs=3) as dpool,
            tc.tile_pool(name="phis", bufs=2) as phpool,
            tc.tile_pool(name="filts", bufs=2) as fspool,
            tc.tile_pool(name="xs", bufs=2) as xpool,
            tc.tile_pool(name="ssel", bufs=4) as spool,
            tc.tile_pool(name="osb", bufs=2) as opool,
            tc.tile_pool(name="pbig", bufs=1, space="PSUM") as pbig,
            tc.tile_pool(name="pfe", bufs=1, space="PSUM") as pfe,
            tc.tile_pool(name="pout", bufs=2, space="PSUM") as pout,
        ):
            idx_sb = cpool.tile([128, nhalf * W], i16)
            nc.sync.dma_start(idx_sb[:], idx_d[:])
            seg_sb = cpool.tile([128, nhalf * nst], fp32)
            nc.sync.dma_start(seg_sb[:], seg_d[:])
            ones_sb = cpool.tile([1, M], fp16)
            nc.sync.dma_start(ones_sb[:], ones_d[:])
            ksc_sb = cpool.tile([M, 1], fp32)
            nc.sync.dma_start(ksc_sb[:], ksc_d[:])
            kbi_sb = cpool.tile([M, 1], fp32)
            nc.sync.dma_start(kbi_sb[:], kbi_d[:])
            C_sb = cpool.tile([M, D], fp16)
            nc.sync.dma_start(C_sb[:], C_d[:])
            ident_sb = cpool.tile([128, 128], fp16)
            nc.sync.dma_start(ident_sb[:], ident_d[:])
            iota_sb = cpool.tile([128, ATB], fp16)
            nc.sync.dma_start(iota_sb[:], iota_d[:])

            for blk in range(nblk):
                out_ps = pout.tile([128, ATB], fp32, tag="outp")
                for half in range(2):
                    h = blk * 2 + half
                    src = tbl_lo if half == 0 else tbl_hi
                    g = gpool.tile([128, nst * D], fp16, tag="g")
                    nc.gpsimd.dma_gather(
                        out_ap=g[:].rearrange("p (n d) -> p n d", d=D),
                        in_ap=src[:],
                        idxs_ap=idx_sb[:, h * W:(h + 1) * W],
                        num_idxs=slots_half,
                        num_idxs_reg=slots_half,
                        elem_size=D,
                        single_packet=False,
                    )
                    dh = dpool.tile([1, slots_half], fp16, tag="dh")
                    nc.sync.dma_start(dh[:], dhi_d[h:h + 1, :])
                    dl = dpool.tile([1, slots_half], fp16, tag="dl")
                    nc.sync.dma_start(dl[:], dlo_d[h:h + 1, :])

                    phi_ps = pbig.tile([128, slots_half], fp32, tag="big")
                    for c0 in range(0, slots_half, 512):
                        c1 = min(c0 + 512, slots_half)
                        nc.tensor.matmul(phi_ps[:M, c0:c1], ones_sb[:],
                                         dh[:, c0:c1], start=True, stop=False)
                        nc.tensor.matmul(phi_ps[:M, c0:c1], ones_sb[:],
                                         dl[:, c0:c1], start=False, stop=True)
                    phi_sb = phpool.tile([128, slots_half], fp16, tag="phi")
                    nc.scalar.activation(
                        phi_sb[:M, :], phi_ps[:M, :],
                        mybir.ActivationFunctionType.Sigmoid,
                        bias=kbi_sb[:], scale=ksc_sb[:])

                    filt_ps = pbig.tile([128, slots_half], fp32, tag="big")
                    for c0 in range(0, slots_half, 512):
                        c1 = min(c0 + 512, slots_half)
                        nc.tensor.matmul(filt_ps[:, c0:c1], C_sb[:],
                                         phi_sb[:M, c0:c1], start=True, stop=True)
                    filt_sb = fspool.tile([128, slots_half], fp16, tag="filt")
                    nc.scalar.activation(filt_sb[:], filt_ps[:],
                                         mybir.ActivationFunctionType.Copy)

                    fe_ps = pfe.tile([128, slots_half], fp16, tag="fe")
                    for j in range(nst):
                        nc.tensor.transpose(fe_ps[:, j * 128:(j + 1) * 128],
                                            filt_sb[:, j * 128:(j + 1) * 128],
                                            ident_sb[:])
                    x_sb = xpool.tile([128, nst * D], fp16, tag="x")
                    nc.vector.tensor_mul(x_sb[:], g[:], fe_ps[:])

                    for j in range(nst):
                        S_sb = spool.tile([128, ATB], fp16, tag="S")
                        nc.vector.tensor_scalar(
                            out=S_sb[:], in0=iota_sb[:],
                            scalar1=seg_sb[:, h * nst + j:h * nst + j + 1],
                            scalar2=None,
                            op0=mybir.AluOpType.is_equal)
                        nc.tensor.matmul(
                            out_ps[:], x_sb[:, j * D:(j + 1) * D], S_sb[:],
                            start=(half == 0 and j == 0),
                            stop=(half == 1 and j == nst - 1),
                            skip_group_check=True)
                o_sb = opool.tile([128, ATB], fp32, tag="o")
                nc.vector.tensor_copy(o_sb[:], out_ps[:])
                nc.sync.dma_start(out_d[:, blk * ATB:(blk + 1) * ATB], o_sb[:])
    nc.compile()
    return nc


def _get_nc(slots_half, nblk):
    key = (slots_half, nblk)
    if key not in _cache:
        _cache[key] = _build_nc(slots_half, nblk)
    return _cache[key]


def _plan_blocks(seg, is_lo, core_edge0, core_edge1):
    """Greedy block packing for one core's edge range [core_edge0, core_edge1).
    A block closes when either half would exceed SLOTS_HALF edges or the
    dest-atom span would exceed ATB. Returns list of (e0, e1, a0)."""
    n = core_edge1 - core_edge0
    segc = seg[core_edge0:core_edge1]
    lo = is_lo[core_edge0:core_edge1]
    cumlo = np.cumsum(lo)            # inclusive count of lo edges in [0..i]
    cumhi = np.arange(1, n + 1) - cumlo
    blocks = []
    start = 0
    while start < n:
        base_lo = cumlo[start - 1] if start > 0 else 0
        base_hi = cumhi[start - 1] if start > 0 else 0
        a0 = int(segc[start])
        # last index (exclusive) we can extend to under each constraint
        e_lo = np.searchsorted(cumlo, base_lo + SLOTS_HALF, side="right")
        e_hi = np.searchsorted(cumhi, base_hi + SLOTS_HALF, side="right")
        e_sp = np.searchsorted(segc, a0 + ATB, side="left")
        end = min(e_lo, e_hi, e_sp, n)
        assert end > start
        blocks.append((core_edge0 + start, core_edge0 + end, a0))
        start = end
    return blocks


def kernel(atom_features, distances, idx_j, seg_i, centers, gamma,
           W1, b1, W2, b2):
    from concourse.bass_utils import run_bass_kernel_spmd

    atom_features = np.asarray(atom_features, np.float32)
    distances = np.asarray(distances, np.float32)
    idx_j = np.asarray(idx_j, np.int32)
    seg_i = np.asarray(seg_i, np.int32)
    centers = np.asarray(centers, np.float32)
    gamma = np.asarray(gamma, np.float32)
    W1 = np.asarray(W1, np.float32)
    b1 = np.asarray(b1, np.float32)
    W2 = np.asarray(W2, np.float32)
    b2 = np.asarray(b2, np.float32)

    ksc, kbi, C, fit_err = _fit_basis(distances, centers, gamma, W1, b1, W2, b2)

    feat16 = atom_features.astype(np.float16)
    d16 = distances.astype(np.float16)
    dlo16 = (distances - d16.astype(np.float32)).astype(np.float16)
    is_lo = idx_j < SPLIT

    # --- core boundaries: atom-aligned, edge-balanced ---
    core_edges = [0]
    for c in range(1, N_CORES):
        target = (E * c) // N_CORES
        a = seg_i[target]
        # first edge of atom a (keep all edges of an atom on one side)
        core_edges.append(int(np.searchsorted(seg_i, a, side="left")))
    core_edges.append(E)
    core_atom0 = [int(seg_i[core_edges[c]]) if core_edges[c] < E else NAT
                  for c in range(N_CORES)]

    # --- greedy block plan per core ---
    plans = [_plan_blocks(seg_i, is_lo, core_edges[c], core_edges[c + 1])
             for c in range(N_CORES)]
    nblk = max(len(p) for p in plans)
    nst = SLOTS_HALF // 128
    nhalf = 2 * nblk
    Wc = SLOTS_HALF // 16

    nc = _get_nc(SLOTS_HALF, nblk)

    in_maps = []
    for c in range(N_CORES):
        idx_arr = np.zeros((nhalf, SLOTS_HALF), np.int16)
        seg_arr = np.full((128, nhalf * nst), -1.0, np.float32)
        dhi_arr = np.zeros((nhalf, SLOTS_HALF), np.float16)
        dlo_arr = np.zeros((nhalf, SLOTS_HALF), np.float16)
        for b, (e0, e1, a0) in enumerate(plans[c]):
            el = np.arange(e0, e1)
            for half in range(2):
                h = 2 * b + half
                m = el[is_lo[el] if half == 0 else ~is_lo[el]]
                pos = np.arange(len(m))
                src = idx_j[m] - (0 if half == 0 else SPLIT)
                idx_arr[h, pos] = src.astype(np.int16)
                seg_arr[pos % 128, h * nst + pos // 128] = (
                    seg_i[m] - a0).astype(np.float32)
                dhi_arr[h, pos] = d16[m]
                dlo_arr[h, pos] = dlo16[m]
        idx_wrap = np.ascontiguousarray(
            idx_arr.reshape(nhalf, Wc, 16).transpose(2, 0, 1)
            .reshape(16, nhalf * Wc))
        idx_wrap = np.tile(idx_wrap, (8, 1))
        in_maps.append({
            "tbl_lo": feat16[:SPLIT],
            "tbl_hi": feat16[SPLIT:],
            "idx": idx_wrap,
            "seg": seg_arr,
            "dhi": dhi_arr,
            "dlo": dlo_arr,
            "ones": np.ones((1, M), np.float16),
            "ksc": ksc.reshape(M, 1),
            "kbi": kbi.reshape(M, 1),
            "C": C.astype(np.float16),
            "ident": np.eye(128, dtype=np.float16),
            "iota": np.tile(np.arange(ATB, dtype=np.float16), (128, 1)),
        })

    res = run_bass_kernel_spmd(nc, in_maps, list(range(N_CORES)))
    out = np.zeros((NAT, D), np.float32)
    for c in range(N_CORES):
        r = res.results[c]["out"]          # [D, nblk*ATB]
        for b, (e0, e1, a0) in enumerate(plans[c]):
            hi = min(a0 + ATB, NAT)
            out[a0:hi] += r[:, b * ATB:b * ATB + (hi - a0)].T
    return out


# revision 3
# speedup vs baseline: 1.0672x; 1.0397x over previous
"""Trainium2 Bass kernel for ContinuousFilterConvolution (SchNet cfconv).

out[a, :] = sum_{e: seg_i[e]=a} filters(d_e) * atom_features[idx_j[e], :]
filters(d) = ssp(ssp(rbf(d) @ W1 + b1) @ W2 + b2), ssp = softplus - log 2.

Strategy (8 NeuronCores, full inputs in / full output out):
- seg_i is sorted, so cores take contiguous atom ranges balanced by EDGE
  count (atom-aligned cuts), eliminating cross-core reduction.
- Per core, edges are greedily packed into blocks: a block closes when
  either source-half reaches SLOTS_HALF edges or its dest-atom span would
  exceed ATB=128.  Blocks are edge-balanced (few % padding) instead of
  atom-aligned (13% padding) -- the kernel is gpsimd descriptor-generation
  bound (~8.2 ns per gathered row), so padding is pure loss.
- Edges are split by source-atom half (idx_j < 25024 vs >=) so gather
  indices fit int16; atom_features rows (fp16) are gathered edge-major by
  nc.gpsimd.dma_gather.
- filters(d) is a 1-D function of distance: approximated by a sigmoid-basis
  expansion fitted on the host from the runtime weights (max err ~4e-5 vs
  an output scale of ~0.14). On-device: PE broadcast-matmul of d (hi+lo fp16
  split) -> ACT sigmoid with per-partition scale/bias -> PE basis matmul
  -> PE transposes to edge-major.
- x = feat * filters on DVE; scatter-add via PE matmul with one-hot
  selection matrices (built by iota-compare on DVE) accumulating into a
  per-block PSUM tile; per-block copy-out to a disjoint DRAM scratch,
  reassembled (with overlap adds at block seams) on the host.
"""
import numpy as np

N_CORES = 8
NAT = 50000
E = 800000
D = 128
ATB = 256             # max dest-atom span of a block (S one-hot columns)
SPLIT = 25024         # source-atom half split (int16 index limit)
M = 64                # basis size (63 sigmoids + 1 const)
SLOTS_HALF = 1152     # slots per (block, half); multiple of 128

_cache = {}


def _fit_basis(distances, centers, gamma, W1, b1, W2, b2):
    """Fit filters(d) ~ C.T @ sigmoid(d*s + b) on the host. Returns
    (scale [M], bias [M], C [M, D] fp32, fit report)."""
    dmin = float(distances.min())
    dmax = float(distances.max())
    span = max(dmax - dmin, 1e-6)
    t = np.linspace(dmin - 0.05 * span, dmax + 0.05 * span, M - 1).astype(np.float64)
    w = (t[1] - t[0])
    scale = np.full(M, 1.0 / w, np.float64)
    bias = -t / w
    # constant basis element
    scale = np.concatenate([scale[: M - 1], [0.0]])
    bias = np.concatenate([bias[: M - 1], [20.0]])

    dg = np.linspace(dmin, dmax, 8192).astype(np.float64)

    def F(d):
        e = np.exp(-gamma[None, :].astype(np.float64)
                   * (d[:, None] - centers[None, :].astype(np.float64)) ** 2)
        h1 = np.logaddexp(0, e @ W1.astype(np.float64) + b1) - np.log(2.0)
        return np.logaddexp(0, h1 @ W2.astype(np.float64) + b2) - np.log(2.0)

    Phi = 1.0 / (1.0 + np.exp(-(dg[:, None] * scale[None, :] + bias[None, :])))
    Y = F(dg)
    C, *_ = np.linalg.lstsq(Phi, Y, rcond=None)
    err = np.abs(Phi @ C - Y).max()
    return (scale.astype(np.float32), bias.astype(np.float32),
            C.astype(np.float32), err)


def _build_nc(slots_half, nblk):
    import concourse.bacc as bacc
    import concourse.mybir as mybir
    import concourse.tile as tile

    fp16 = mybir.dt.float16
    fp32 = mybir.dt.float32
    i16 = mybir.dt.int16

    nhalf = 2 * nblk
    nst = slots_half // 128   # sub-tiles (128 slots) per half
    W = slots_half // 16      # idx columns per half

    nc = bacc.Bacc(None, target_bir_lowering=False, debug=False)
    tbl_lo = nc.declare_dram_parameter("tbl_lo", [SPLIT, D], fp16, isOutput=False)
    tbl_hi = nc.declare_dram_parameter("tbl_hi", [NAT - SPLIT, D], fp16, isOutput=False)
    idx_d = nc.declare_dram_parameter("idx", [128, nhalf * W], i16, isOutput=False)
    seg_d = nc.declare_dram_parameter("seg", [128, nhalf * nst], fp32, isOutput=False)
    dhi_d = nc.declare_dram_parameter("dhi", [nhalf, slots_half], fp16, isOutput=False)
    dlo_d = nc.declare_dram_parameter("dlo", [nhalf, slots_half], fp16, isOutput=False)
    ones_d = nc.declare_dram_parameter("ones", [1, M], fp16, isOutput=False)
    ksc_d = nc.declare_dram_parameter("ksc", [M, 1], fp32, isOutput=False)
    kbi_d = nc.declare_dram_parameter("kbi", [M, 1], fp32, isOutput=False)
    C_d = nc.declare_dram_parameter("C", [M, D], fp16, isOutput=False)
    ident_d = nc.declare_dram_parameter("ident", [128, 128], fp16, isOutput=False)
    iota_d = nc.declare_dram_parameter("iota", [128, ATB], fp16, isOutput=False)
    out_d = nc.declare_dram_parameter("out", [128, nblk * ATB], fp32, isOutput=True)

    with tile.TileContext(nc) as tc:
        with (
            tc.tile_pool(name="const", bufs=1) as cpool,
            tc.tile_pool(name="gat", bufs=4) as gpool,
            tc.tile_pool(name="dist", bufs=3) as dpool,
            tc.tile_pool(name="phis", bufs=2) as phpool,
            tc.tile_pool(name="filts", bufs=2) as fspool,
            tc.tile_pool(name="xs", bufs=2) as xpool,
            tc.tile_pool(name="ssel", bufs=4) as spool,
            tc.tile_pool(name="osb", bufs=2) as opool,
            tc.tile_pool(name="pbig", bufs=1, space="PSUM") as pbig,
            tc.tile_pool(name="pfe", bufs=1, space="PSUM") as pfe,
            tc.tile_pool(name="pout", bufs=2, space="PSUM") as pout,
        ):
            idx_sb = cpool.tile([128, nhalf * W], i16)
            nc.sync.dma_start(idx_sb[:], idx_d[:])
            seg_sb = cpool.tile([128, nhalf * nst], fp32)
            nc.sync.dma_start(seg_sb[:], seg_d[:])
            ones_sb = cpool.tile([1, M], fp16)
            nc.sync.dma_start(ones_sb[:], ones_d[:])
            ksc_sb = cpool.tile([M, 1], fp32)
            nc.sync.dma_start(ksc_sb[:], ksc_d[:])
            kbi_sb = cpool.tile([M, 1], fp32)
            nc.sync.dma_start(kbi_sb[:], kbi_d[:])
            C_sb = cpool.tile([M, D], fp16)
            nc.sync.dma_start(C_sb[:], C_d[:])
            ident_sb = cpool.tile([128, 128], fp16)
            nc.sync.dma_start(ident_sb[:], ident_d[:])
            iota_sb = cpool.tile([128, ATB], fp16)
            nc.sync.dma_start(iota_sb[:], iota_d[:])

            for blk in range(nblk):
                out_ps = pout.tile([128, ATB], fp32, tag="outp")
                for half in range(2):
                    h = blk * 2 + half
                    src = tbl_lo if half == 0 else tbl_hi
                    g = gpool.tile([128, nst * D], fp16, tag="g")
                    nc.gpsimd.dma_gather(
                        out_ap=g[:].rearrange("p (n d) -> p n d", d=D),
                        in_ap=src[:],
                        idxs_ap=idx_sb[:, h * W:(h + 1) * W],
                        num_idxs=slots_half,
                        num_idxs_reg=slots_half,
                        elem_size=D,
                        single_packet=False,
                    )
                    dh = dpool.tile([1, slots_half], fp16, tag="dh")
                    nc.sync.dma_start(dh[:], dhi_d[h:h + 1, :])
                    dl = dpool.tile([1, slots_half], fp16, tag="dl")
                    nc.sync.dma_start(dl[:], dlo_d[h:h + 1, :])

                    phi_ps = pbig.tile([128, slots_half], fp32, tag="big")
                    for c0 in range(0, slots_half, 512):
                        c1 = min(c0 + 512, slots_half)
                        nc.tensor.matmul(phi_ps[:M, c0:c1], ones_sb[:],
                                         dh[:, c0:c1], start=True, stop=False)
                        nc.tensor.matmul(phi_ps[:M, c0:c1], ones_sb[:],
                                         dl[:, c0:c1], start=False, stop=True)
                    phi_sb = phpool.tile([128, slots_half], fp16, tag="phi")
                    nc.scalar.activation(
                        phi_sb[:M, :], phi_ps[:M, :],
                        mybir.ActivationFunctionType.Sigmoid,
                        bias=kbi_sb[:], scale=ksc_sb[:])

                    filt_ps = pbig.tile([128, slots_half], fp32, tag="big")
                    for c0 in range(0, slots_half, 512):
                        c1 = min(c0 + 512, slots_half)
                        nc.tensor.matmul(filt_ps[:, c0:c1], C_sb[:],
                                         phi_sb[:M, c0:c1], start=True, stop=True)
                    filt_sb = fspool.tile([128, slots_half], fp16, tag="filt")
                    nc.scalar.activation(filt_sb[:], filt_ps[:],
                                         mybir.ActivationFunctionType.Copy)

                    fe_ps = pfe.tile([128, slots_half], fp16, tag="fe")
                    for j in range(nst):
                        nc.tensor.transpose(fe_ps[:, j * 128:(j + 1) * 128],
                                            filt_sb[:, j * 128:(j + 1) * 128],
                                            ident_sb[:])
                    x_sb = xpool.tile([128, nst * D], fp16, tag="x")
                    nc.vector.tensor_mul(x_sb[:], g[:], fe_ps[:])

                    for j in range(nst):
                        S_sb = spool.tile([128, ATB], fp16, tag="S")
                        nc.vector.tensor_scalar(
                            out=S_sb[:], in0=iota_sb[:],
                            scalar1=seg_sb[:, h * nst + j:h * nst + j + 1],
                            scalar2=None,
                            op0=mybir.AluOpType.is_equal)
                        nc.tensor.matmul(
                            out_ps[:], x_sb[:, j * D:(j + 1) * D], S_sb[:],
                            start=(half == 0 and j == 0),
                            stop=(half == 1 and j == nst - 1),
                            skip_group_check=True)
                o_sb = opool.tile([128, ATB], fp32, tag="o")
                nc.vector.tensor_copy(o_sb[:], out_ps[:])
                nc.sync.dma_start(out_d[:, blk * ATB:(blk + 1) * ATB], o_sb[:])
    nc.compile()
    return nc


def _get_nc(slots_half, nblk):
    key = (slots_half, nblk)
    if key not in _cache:
        _cache[key] = _build_nc(slots_half, nblk)
    return _cache[key]


def _plan_blocks(seg, is_lo, core_edge0, core_edge1):
    """Greedy block packing for one core's edge range [core_edge0, core_edge1).
    A block closes when either half would exceed SLOTS_HALF edges or the
    dest-atom span would exceed ATB. Returns list of (e0, e1, a0)."""
    n = core_edge1 - core_edge0
    segc = seg[core_edge0:core_edge1]
    lo = is_lo[core_edge0:core_edge1]
    cumlo = np.cumsum(lo)            # inclusive count of lo edges in [0..i]
    cumhi = np.arange(1, n + 1) - cumlo
    blocks = []
    start = 0
    while start < n:
        base_lo = cumlo[start - 1] if start > 0 else 0
        base_hi = cumhi[start - 1] if start > 0 else 0
        a0 = int(segc[start])
        # last index (exclusive) we can extend to under each constraint
        e_lo = np.searchsorted(cumlo, base_lo + SLOTS_HALF, side="right")
        e_hi = np.searchsorted(cumhi, base_hi + SLOTS_HALF, side="right")
        e_sp = np.searchsorted(segc, a0 + ATB, side="left")
        end = min(e_lo, e_hi, e_sp, n)
        assert end > start
        blocks.append((core_edge0 + start, core_edge0 + end, a0))
        start = end
    return blocks


def kernel(atom_features, distances, idx_j, seg_i, centers, gamma,
           W1, b1, W2, b2):
    from concourse.bass_utils import run_bass_kernel_spmd

    atom_features = np.asarray(atom_features, np.float32)
    distances = np.asarray(distances, np.float32)
    idx_j = np.asarray(idx_j, np.int32)
    seg_i = np.asarray(seg_i, np.int32)
    centers = np.asarray(centers, np.float32)
    gamma = np.asarray(gamma, np.float32)
    W1 = np.asarray(W1, np.float32)
    b1 = np.asarray(b1, np.float32)
    W2 = np.asarray(W2, np.float32)
    b2 = np.asarray(b2, np.float32)

    ksc, kbi, C, fit_err = _fit_basis(distances, centers, gamma, W1, b1, W2, b2)

    feat16 = atom_features.astype(np.float16)
    d16 = distances.astype(np.float16)
    dlo16 = (distances - d16.astype(np.float32)).astype(np.float16)
    is_lo = idx_j < SPLIT

    # --- core boundaries: atom-aligned, edge-balanced ---
    core_edges = [0]
    for c in range(1, N_CORES):
        target = (E * c) // N_CORES
        a = seg_i[target]
        # first edge of atom a (keep all edges of an atom on one side)
        core_edges.append(int(np.searchsorted(seg_i, a, side="left")))
    core_edges.append(E)
    core_atom0 = [int(seg_i[core_edges[c]]) if core_edges[c] < E else NAT
                  for c in range(N_CORES)]

    # --- greedy block plan per core ---
    plans = [_plan_blocks(seg_i, is_lo, core_edges[c], core_edges[c + 1])
             for c in range(N_CORES)]
    nblk = max(len(p) for p in plans)
    nst = SLOTS_HALF // 128
    nhalf = 2 * nblk
    Wc = SLOTS_HALF // 16

    nc = _get_nc(SLOTS_HALF, nblk)

    in_maps = []
    for c in range(N_CORES):
        idx_arr = np.zeros((nhalf, SLOTS_HALF), np.int16)
        seg_arr = np.full((128, nhalf * nst), -1.0, np.float32)
        dhi_arr = np.zeros((nhalf, SLOTS_HALF), np.float16)
        dlo_arr = np.zeros((nhalf, SLOTS_HALF), np.float16)
        for b, (e0, e1, a0) in enumerate(plans[c]):
            el = np.arange(e0, e1)
            for half in range(2):
                h = 2 * b + half
                m = el[is_lo[el] if half == 0 else ~is_lo[el]]
                pos = np.arange(len(m))
                src = idx_j[m] - (0 if half == 0 else SPLIT)
                idx_arr[h, pos] = src.astype(np.int16)
                seg_arr[pos % 128, h * nst + pos // 128] = (
                    seg_i[m] - a0).astype(np.float32)
                dhi_arr[h, pos] = d16[m]
                dlo_arr[h, pos] = dlo16[m]
        idx_wrap = np.ascontiguousarray(
            idx_arr.reshape(nhalf, Wc, 16).transpose(2, 0, 1)
            .reshape(16, nhalf * Wc))
        idx_wrap = np.tile(idx_wrap, (8, 1))
        in_maps.append({
            "tbl_lo": feat16[:SPLIT],
            "tbl_hi": feat16[SPLIT:],
            "idx": idx_wrap,
            "seg": seg_arr,
            "dhi": dhi_arr,
            "dlo": dlo_arr,
            "ones": np.ones((1, M), np.float16),
            "ksc": ksc.reshape(M, 1),
            "kbi": kbi.reshape(M, 1),
            "C": C.astype(np.float16),
            "ident": np.eye(128, dtype=np.float16),
            "iota": np.tile(np.arange(ATB, dtype=np.float16), (128, 1)),
        })

    res = run_bass_kernel_spmd(nc, in_maps, list(range(N_CORES)))
    out = np.zeros((NAT, D), np.float32)
    for c in range(N_CORES):
        r = res.results[c]["out"]          # [D, nblk*ATB]
        for b, (e0, e1, a0) in enumerate(plans[c]):
            hi = min(a0 + ATB, NAT)
            out[a0:hi] += r[:, b * ATB:b * ATB + (hi - a0)].T
    return out
